# revision 1
# baseline (speedup 1.0000x reference)
"""Trainium2 Bass kernel for nn_FFT: per-16-float-chunk 4x2 complex FFT.

The reference op reshapes x (B, 32, 256) -> (B, 16, 32, 4, 2, 2), treats the
trailing (4, 2, 2) as a 4x2 complex plane (last axis = re/im), applies a 2D FFT
over the (4, 2) plane, and writes real/imag concatenated:
out idx within each 16-float chunk = 4*k + c + 2*is_imag.

Because all twiddles of a 4x2 FFT are {±1, ±i}, the transform of each chunk of
16 consecutive floats is a fixed signed-add network: 3 butterfly stages,
48 real adds per chunk.  That maps to 10 strided VectorE tensor_tensor ops per
SBUF tile (multi-dim access patterns process all chunks of a tile per op).

Sharding: batch dim across 8 cores (2048 samples each); per core the data is a
flat contiguous block of 16,777,216 floats viewed as (128 partitions, 131072),
streamed through SBUF in 16 tiles of (128, 8192) with triple buffering.
Loads are issued on the SP HWDGE ring (nc.sync), stores on the ACT HWDGE ring
(nc.scalar); the rings drain independently, which roughly doubles streaming
bandwidth vs one ring (measured ~330 GB/s/core aggregate, ~92% of the per-core
HBM limit).
"""

import numpy as np

B, H, W = 16384, 32, 256
N_CORES = 8
PER_CORE = B // N_CORES                # 2048 samples
FLAT = PER_CORE * H * W                # 16,777,216 floats per core
P = 128
FREE = FLAT // P                       # 131072 floats per partition
F_TILE = 8192                          # floats per partition per tile
N_TILES = FREE // F_TILE               # 16

_CACHED_NC = None


def _butterfly_tile(nc, tin, tout):
    """Apply the 4x2 FFT butterfly network to one SBUF tile.

    tin holds the input tile; tout receives the final result.  Intermediates
    ping-pong: stage A writes tout (s/d), stage B writes tin (t), stage C
    writes tout (final, with the re/im output permutation folded into the
    destination access patterns).
    Chunk element index: idx = 4*r + 2*c + ri  (r=row 0..3, c=col 0..1, ri=re/im).
    """
    v = nc.vector
    i5 = tin[:].rearrange("p (n r c i) -> p n r c i", r=4, c=2, i=2)
    o5 = tout[:].rearrange("p (n r c i) -> p n r c i", r=4, c=2, i=2)

    # Stage A (length-2 FFT over c):  s[r] = x[r,0]+x[r,1] -> (r, c=0, ri)
    #                                 d[r] = x[r,0]-x[r,1] -> (r, c=1, ri)
    v.tensor_add(out=o5[:, :, :, 0, :], in0=i5[:, :, :, 0, :], in1=i5[:, :, :, 1, :])
    v.tensor_sub(out=o5[:, :, :, 1, :], in0=i5[:, :, :, 0, :], in1=i5[:, :, :, 1, :])

    # Stage B (first level of FFT4 over r): y -> t where
    # t[2b] = y[b] + y[b+2], t[2b+1] = y[b] - y[b+2], b in {0,1};
    # m = 2*c + ri runs over the 4 trailing values of each row slot.
    sB = tout[:].rearrange("p (n qh ql m) -> p n qh ql m", qh=2, ql=2, m=4)
    tB = tin[:].rearrange("p (n qh ql m) -> p n qh ql m", qh=2, ql=2, m=4)
    v.tensor_add(out=tB[:, :, :, 0, :], in0=sB[:, :, 0, :, :], in1=sB[:, :, 1, :, :])
    v.tensor_sub(out=tB[:, :, :, 1, :], in0=sB[:, :, 0, :, :], in1=sB[:, :, 1, :, :])

    # Stage C (second level of FFT4, twiddle -i on the odd branch), writing the
    # reference's output layout: out idx = 4*k + 2*ri + c.
    t5 = tin[:].rearrange("p (n q c i) -> p n q c i", q=4, c=2, i=2)
    f5 = tout[:].rearrange("p (n k i2 c2) -> p n k i2 c2", k=4, i2=2, c2=2)
    # F0 = t0 + t2 ; F2 = t0 - t2   (srcs transposed (c,i)->(i,c) to match dst order)
    t0 = t5[:, :, 0, :, :].transpose([0, 1, 3, 2])
    t2 = t5[:, :, 2, :, :].transpose([0, 1, 3, 2])
    v.tensor_add(out=f5[:, :, 0, :, :], in0=t0, in1=t2)
    v.tensor_sub(out=f5[:, :, 2, :, :], in0=t0, in1=t2)
    # F1 = t1 - i*t3 ; F3 = t1 + i*t3
    t1re = t5[:, :, 1, :, 0]
    t1im = t5[:, :, 1, :, 1]
    t3re = t5[:, :, 3, :, 0]
    t3im = t5[:, :, 3, :, 1]
    v.tensor_add(out=f5[:, :, 1, 0, :], in0=t1re, in1=t3im)   # F1.re = t1.re + t3.im
    v.tensor_sub(out=f5[:, :, 1, 1, :], in0=t1im, in1=t3re)   # F1.im = t1.im - t3.re
    v.tensor_sub(out=f5[:, :, 3, 0, :], in0=t1re, in1=t3im)   # F3.re = t1.re - t3.im
    v.tensor_add(out=f5[:, :, 3, 1, :], in0=t1im, in1=t3re)   # F3.im = t1.im + t3.re


def _build(reps: int = 1):
    from concourse import bacc
    import concourse.mybir as mybir
    from concourse.tile import TileContext

    nc = bacc.Bacc("TRN2", target_bir_lowering=False, debug=False)
    x = nc.dram_tensor("x", (P, FREE), mybir.dt.float32, kind="ExternalInput").ap()
    y = nc.dram_tensor("y", (P, FREE), mybir.dt.float32, kind="ExternalOutput").ap()

    H_TILE = F_TILE // 4
    with TileContext(nc) as tc:
        with tc.tile_pool(name="pool", bufs=3) as pool:
            for _ in range(reps):
                for j in range(N_TILES):
                    lo = j * F_TILE
                    tin = pool.tile([P, F_TILE], mybir.dt.float32, tag="tin")
                    # Loads on the SP HWDGE ring, stores on the ACT HWDGE ring:
                    # the two rings drain independently, nearly doubling
                    # streaming bandwidth vs a single ring.  Four 1MB DMAs per
                    # direction per tile interleave best with the compute
                    # cadence (A/B-measured -34us/iter vs two 2MB DMAs).
                    for s in range(4):
                        nc.sync.dma_start(
                            out=tin[:, s * H_TILE:(s + 1) * H_TILE],
                            in_=x[:, lo + s * H_TILE:lo + (s + 1) * H_TILE],
                        )
                    tout = pool.tile([P, F_TILE], mybir.dt.float32, tag="tout")
                    _butterfly_tile(nc, tin, tout)
                    for s in range(4):
                        nc.scalar.dma_start(
                            out=y[:, lo + s * H_TILE:lo + (s + 1) * H_TILE],
                            in_=tout[:, s * H_TILE:(s + 1) * H_TILE],
                        )
    nc.compile()
    nc.finalize()
    return nc


def get_nc():
    global _CACHED_NC
    if _CACHED_NC is None:
        _CACHED_NC = _build()
    return _CACHED_NC


def kernel(x: np.ndarray, **_unused) -> np.ndarray:
    from concourse.bass_utils import run_bass_kernel_spmd

    x = np.ascontiguousarray(np.asarray(x, dtype=np.float32))
    assert x.shape == (B, H, W), x.shape
    nc = get_nc()
    xs = x.reshape(N_CORES, P, FREE)
    in_maps = [{"x": xs[i]} for i in range(N_CORES)]
    res = run_bass_kernel_spmd(nc, in_maps, core_ids=list(range(N_CORES)))
    out = np.stack([r["y"] for r in res.results])
    return out.reshape(B, H, W)

